# revision 7
# baseline (speedup 1.0000x reference)
"""Trainium2 Bass kernel for a 2-layer GraphConv block (PyG GraphConv, aggr=add):
    h1  = leaky_relu(segsum(x[src], dst) @ W1_rel.T + b1 + x @ W1_root.T)
    out = leaky_relu(segsum(h1[src], dst) @ W2_rel.T + b2 + h1 @ W2_root.T + x)

Self-contained: takes full inputs, shards nodes (dst) across 8 NeuronCores,
runs one SPMD Bass program, returns the full output.

Aggregation strategy (v2): no DMA scatter-add. Edges are grouped by
(src part q, dst chunk c of 128 rows), gathered with swdge dma_gather into
SBUF tiles of 128 tokens, and summed into per-chunk PSUM accumulators with
PE matmuls against one-hot matrices S[j, m] = (dst_rel[j] == m) generated
on-chip by a DVE is_equal against an iota constant. Group sizes are padded
to the max across cores so the instruction stream is uniform (SPMD).
Each chunk's accumulation matmuls run consecutively into a bank-sized PSUM
accumulator (PSUM start=True zeroes a whole 2KB region, so groups cannot
interleave within a bank).
"""
import sys

sys.path.insert(0, '/opt/trn_rl_repo')

import numpy as np

N = 100000
D = 64
NCORES = 8
NPART = N // NCORES            # 12500
NP = 12544                     # 98 * 128, padded part size
NCHUNK = NP // 128             # 98
SBC = 8                        # chunks per superblock
NSB = (NCHUNK + SBC - 1) // SBC
NEG_SLOPE = 0.01


def _make_plan(src, dst):
    """Uniform SPMD plan + per-core index/dst streams.

    Gather-stream order: superblock sb -> src part q -> chunk c; group (q, c)
    holds K[q, c] = ceil(max_core_count / 128) tiles of 128 tokens.
    Returns (plan, gstreams, dstreams):
      plan: dict(L, T, K, group_tile0, calls)
        calls[sb][q] = (off_tokens, ntok, tile_lo) or None
      gstreams: int16 [NCORES, L] gather idx into part-q tensor (pad 0)
      dstreams: float32 [NCORES, 128, T] dst-rel per token (pad -1)
    """
    per_core = []
    counts = np.zeros((NCORES, NCORES, NCHUNK), np.int64)
    for p in range(NCORES):
        sel = (dst >= p * NPART) & (dst < (p + 1) * NPART)
        s = src[sel]
        dloc = (dst[sel] - p * NPART).astype(np.int64)
        q = s // NPART
        sl = (s - q * NPART).astype(np.int64)
        c = dloc // 128
        dr = dloc - c * 128
        np.add.at(counts, (p, q, c), 1)
        per_core.append((q, c, sl, dr))

    K = (counts.max(axis=0) + 127) // 128          # [q, c] tiles per group

    # stream layout
    ntiles = 0
    group_tile0 = np.zeros((NCORES, NCHUNK), np.int64)
    calls = []
    for sb in range(NSB):
        c0, c1 = sb * SBC, min((sb + 1) * SBC, NCHUNK)
        sb_calls = []
        for q in range(NCORES):
            tlo = ntiles
            for c in range(c0, c1):
                group_tile0[q, c] = ntiles
                ntiles += int(K[q, c])
            nt = ntiles - tlo
            sb_calls.append((tlo * 128, nt * 128, tlo) if nt else None)
        calls.append(sb_calls)
    T = ntiles
    L = T * 128

    gstreams = np.zeros((NCORES, L), np.int16)
    dstreams = np.full((NCORES, 128, T), -1.0, np.float32)
    for p, (q, c, sl, dr) in enumerate(per_core):
        o = np.lexsort((sl, c, q))   # src-sorted within groups for DRAM locality
        q2, c2, sl2, dr2 = q[o], c[o], sl[o], dr[o]
        key = q2 * NCHUNK + c2
        nb = np.r_[True, key[1:] != key[:-1]]
        gid = np.cumsum(nb) - 1
        st = np.flatnonzero(nb)
        off_in_grp = np.arange(len(key)) - st[gid]
        base_tile = group_tile0[q2[st], c2[st]][gid]
        posn = base_tile * 128 + off_in_grp
        gstreams[p][posn] = sl2.astype(np.int16)
        dstreams[p][posn % 128, posn // 128] = dr2

    plan = dict(L=L, T=T, K=K, group_tile0=group_tile0, calls=calls)
    return plan, gstreams, dstreams


def _wrap_stream(a):
    """[L] int16 -> [128, L//16] wrapped (idx i at [i%16, i//16]) replicated 8x."""
    L = len(a)
    assert L % 16 == 0
    w = a.reshape(L // 16, 16).T  # [16, cols]
    return np.tile(w, (8, 1)).copy()


def _build_nc(plan):
    from concourse import tile, mybir, masks
    import concourse.bacc as bacc

    L, T = plan["L"], plan["T"]
    K = plan["K"]
    group_tile0 = plan["group_tile0"]
    cols = L // 16
    f32 = mybir.dt.float32
    bf16 = mybir.dt.bfloat16
    i16 = mybir.dt.int16

    # max tiles per gather call (for SBUF gather-buffer sizing)
    max_call_tiles = max(c[1] // 128 for sbc in plan["calls"] for c in sbc if c)
    max_sb_cols = max(sum(c[1] for c in sbc if c) // 16 for sbc in plan["calls"])

    nc = bacc.Bacc(None, target_bir_lowering=False, num_devices=NCORES,
                   dynamic_dma_scratch_size=16384, num_swdge_queues=4)

    x_parts = [nc.declare_dram_parameter(f"x_part{q}", [NP, D], f32, isOutput=False)
               for q in range(NCORES)]
    xT_in = nc.declare_dram_parameter("xT", [D, NP], f32, isOutput=False)
    w_ins = {}
    for nm in ["W1relT", "W1rootT", "W2relT", "W2rootT"]:
        w_ins[nm] = nc.declare_dram_parameter(nm, [D, D], f32, isOutput=False)
    b_ins = {nm: nc.declare_dram_parameter(nm, [1, D], f32, isOutput=False)
             for nm in ["b1", "b2"]}
    gidx_in = nc.declare_dram_parameter("gidx", [128, cols], i16, isOutput=False)
    dcol_in = nc.declare_dram_parameter("dcol", [128, T], f32, isOutput=False)
    ndcol_in = nc.declare_dram_parameter("ndcol", [128, T], f32, isOutput=False)
    iota_in = nc.declare_dram_parameter("iota", [128, 128], f32, isOutput=False)
    y_out = nc.declare_dram_parameter("y", [NP, D], f32, isOutput=True)

    h1_bounce = nc.dram_tensor("h1_bounce", [NP, D], f32)
    h_full = nc.dram_tensor("h_full", [NCORES * NP, D], f32, addr_space="Shared")

    with tile.TileContext(nc) as tc:
        with (
            tc.tile_pool(name="const", bufs=1) as cpool,
            tc.tile_pool(name="gi", bufs=2) as ipool,
            tc.tile_pool(name="gbuf", bufs=2) as gpool,
            tc.tile_pool(name="g16", bufs=2) as g16pool,
            tc.tile_pool(name="sgen", bufs=6) as spool,
            tc.tile_pool(name="mm", bufs=3) as mpool,
            tc.tile_pool(name="acc", bufs=3, space="PSUM") as apool,
            tc.tile_pool(name="psmall", bufs=2, space="PSUM") as ppool,
        ):
            # ---- constants ----
            ident = cpool.tile([128, 128], f32)
            masks.make_identity(nc, ident[:])
            ones1 = cpool.tile([1, 128], f32)
            nc.gpsimd.memset(ones1[:], 1.0)
            iota_t = cpool.tile([128, 128], f32)
            nc.sync.dma_start(iota_t[:], iota_in[:])
            wt = {}
            for nm, t_in in w_ins.items():
                t = cpool.tile([D, D], f32, tag=nm)
                nc.sync.dma_start(t[:], t_in[:])
                wt[nm] = t
            bt = {}
            for nm, t_in in b_ins.items():
                t = cpool.tile([1, D], f32, tag=nm)
                nc.sync.dma_start(t[:], t_in[:])
                bt[nm] = t

            # ---- resident streams / features ----
            dcol = cpool.tile([128, T], f32)
            nc.sync.dma_start(dcol[:], dcol_in[:])
            ndcol = cpool.tile([128, T], f32)
            nc.sync.dma_start(ndcol[:], ndcol_in[:])
            xT_sb = cpool.tile([D, NP], f32)
            for a in range(0, NP, 4096):
                n = min(4096, NP - a)
                nc.sync.dma_start(xT_sb[:, a:a + n], xT_in[:, a:a + n])
            h1T_sb = cpool.tile([D, NP], f32)

            qn = [0]

            def layer(src_aps, w_rel, w_root, bias, root_sb, residual,
                      out_rows, save_h1T):
                for sb in range(NSB):
                    c0, c1 = sb * SBC, min((sb + 1) * SBC, NCHUNK)
                    metas = [m for m in plan["calls"][sb] if m is not None]
                    sb_off = min(m[0] for m in metas)
                    sb_end = max(m[0] + m[1] for m in metas)
                    gi = ipool.tile([128, max_sb_cols], i16, tag="gi")
                    nc.sync.dma_start(gi[:, :(sb_end - sb_off) // 16],
                                      gidx_in[:, sb_off // 16:sb_end // 16])
                    bufs = {}
                    for q in range(NCORES):
                        meta = plan["calls"][sb][q]
                        if meta is None:
                            continue
                        off, ntok, tlo = meta
                        rows = ntok // 128
                        gb = gpool.tile([128, max_call_tiles, D], f32, tag="gb")
                        # swdge gather calls are capped at 1024 idxs (64
                        # descs/engine single-packet limit) -- split.
                        for a in range(0, rows, 8):
                            r = min(8, rows - a)
                            o16 = (off - sb_off) // 16
                            nc.gpsimd.dma_gather(
                                gb[:, a:a + r, :], src_aps[q],
                                gi[:, o16 + a * 8:o16 + (a + r) * 8],
                                r * 128, r * 128, D, queue_num=qn[0] % 4)
                            qn[0] += 1
                        gb16 = g16pool.tile([128, max_call_tiles, D], bf16,
                                            tag=f"g16_{q}")
                        nc.vector.tensor_copy(gb16[:, :rows, :], gb[:, :rows, :])
                        bufs[q] = (gb16, tlo)
                    for c in range(c0, c1):
                        qlist = [q for q in range(NCORES)
                                 if q in bufs and K[q, c] > 0]
                        acc = apool.tile([128, D], f32, tag="acc")
                        nmm = sum(int(K[q, c]) for q in qlist)
                        i = 0
                        for q in qlist:
                            gb16, tlo = bufs[q]
                            t0 = int(group_tile0[q, c])
                            for k in range(int(K[q, c])):
                                t = t0 + k
                                S = spool.tile([128, 128], bf16, tag="S")
                                if t % 2 == 0:
                                    nc.vector.tensor_scalar(
                                        S[:], iota_t[:], dcol[:, t:t + 1], None,
                                        op0=mybir.AluOpType.is_equal)
                                else:
                                    sa = spool.tile([128, 128], f32, tag="sabs")
                                    nc.scalar.activation(
                                        sa[:], iota_t[:],
                                        mybir.ActivationFunctionType.Abs,
                                        bias=ndcol[:, t:t + 1])
                                    nc.scalar.activation(
                                        S[:], sa[:],
                                        mybir.ActivationFunctionType.Relu,
                                        bias=1.0, scale=-1.0)
                                nc.tensor.matmul(
                                    acc[:], S[:], gb16[:, t - tlo, :],
                                    start=(i == 0), stop=(i == nmm - 1))
                                i += 1
                        # dense tail for chunk c
                        agg_sb = mpool.tile([128, D], f32, tag="agg")
                        nc.vector.tensor_copy(agg_sb[:], acc[:])
                        ps_t = ppool.tile([D, 128], f32, tag="ps_t")
                        nc.tensor.matmul(ps_t[:], agg_sb[:], ident[:],
                                         is_transpose=True)
                        aT = mpool.tile([D, 128], f32, tag="aT")
                        nc.vector.tensor_copy(aT[:], ps_t[:])
                        po = ppool.tile([128, D], f32, tag="po")
                        nc.tensor.matmul(po[:], aT[:], w_rel[:],
                                         start=True, stop=False)
                        nc.tensor.matmul(po[:], root_sb[:, 128 * c:128 * (c + 1)],
                                         w_root[:], start=False, stop=False)
                        if residual:
                            nc.tensor.matmul(po[:], xT_sb[:, 128 * c:128 * (c + 1)],
                                             ident[:D, :D], start=False, stop=False)
                        nc.tensor.matmul(po[:], ones1[:], bias[:],
                                         start=False, stop=True)
                        tmp = mpool.tile([128, D], f32, tag="tmp")
                        nc.vector.tensor_scalar_mul(tmp[:], po[:], NEG_SLOPE)
                        hrow = mpool.tile([128, D], f32, tag="hrow")
                        nc.vector.tensor_max(hrow[:], po[:], tmp[:])
                        nc.sync.dma_start(out_rows[128 * c:128 * (c + 1), :],
                                          hrow[:])
                        if save_h1T:
                            ps_h = ppool.tile([D, 128], f32, tag="ps_t")
                            nc.tensor.matmul(ps_h[:], hrow[:], ident[:],
                                             is_transpose=True)
                            nc.vector.tensor_copy(h1T_sb[:, 128 * c:128 * (c + 1)],
                                                  ps_h[:])

            # ================= layer 1 =================
            layer([xp[:] for xp in x_parts], wt["W1relT"], wt["W1rootT"],
                  bt["b1"], xT_sb, False, h1_bounce, True)

            # ================= halo exchange =================
            nc.gpsimd.collective_compute(
                "AllGather", mybir.AluOpType.bypass,
                replica_groups=[list(range(NCORES))],
                ins=[h1_bounce[:].opt()], outs=[h_full[:].opt()])

            # ================= layer 2 =================
            layer([h_full[q * NP:(q + 1) * NP, :] for q in range(NCORES)],
                  wt["W2relT"], wt["W2rootT"], bt["b2"], h1T_sb, True,
                  y_out, False)

    nc.compile()
    return nc


def _prep_inputs(x, edge_index, W1_rel, b1, W1_root, W2_rel, b2, W2_root):
    src = np.asarray(edge_index[0]).astype(np.int64)
    dst = np.asarray(edge_index[1]).astype(np.int64)
    plan, gstreams, dstreams = _make_plan(src, dst)

    x = np.asarray(x, np.float32)
    xp_all = []
    for q in range(NCORES):
        xp = np.zeros((NP, D), np.float32)
        xp[:NPART] = x[q * NPART:(q + 1) * NPART]
        xp_all.append(xp)

    common = {f"x_part{q}": xp_all[q] for q in range(NCORES)}
    common["W1relT"] = np.ascontiguousarray(np.asarray(W1_rel, np.float32).T)
    common["W1rootT"] = np.ascontiguousarray(np.asarray(W1_root, np.float32).T)
    common["W2relT"] = np.ascontiguousarray(np.asarray(W2_rel, np.float32).T)
    common["W2rootT"] = np.ascontiguousarray(np.asarray(W2_root, np.float32).T)
    common["b1"] = np.asarray(b1, np.float32).reshape(1, D)
    common["b2"] = np.asarray(b2, np.float32).reshape(1, D)
    common["iota"] = np.tile(np.arange(128, dtype=np.float32)[None, :],
                             (128, 1))

    in_maps = []
    for p in range(NCORES):
        m = dict(common)
        m["xT"] = np.ascontiguousarray(xp_all[p].T)
        m["gidx"] = _wrap_stream(gstreams[p])
        m["dcol"] = dstreams[p]
        m["ndcol"] = -dstreams[p]
        in_maps.append(m)
    return plan, in_maps


def kernel(x, edge_index, W1_rel, b1, W1_root, W2_rel, b2, W2_root):
    from concourse import bass_utils

    plan, in_maps = _prep_inputs(x, edge_index, W1_rel, b1, W1_root,
                                 W2_rel, b2, W2_root)
    nc = _build_nc(plan)
    res = bass_utils.run_bass_kernel_spmd(nc, in_maps, core_ids=list(range(NCORES)))
    out = np.concatenate([res.results[p]["y"][:NPART] for p in range(NCORES)], 0)
    return out.astype(np.float32)


if __name__ == "__main__":
    # host-side plan self-check in numpy (no device)
    rng = np.random.default_rng(0)
    E = 400000
    src = rng.integers(0, N, E)
    dst = rng.integers(0, N, E)
    plan, gstreams, dstreams = _make_plan(src, dst)
    mx = max(c[1] // 128 for sbc in plan["calls"] for c in sbc if c)
    print(f"L={plan['L']} T={plan['T']} maxcall_tiles={mx}")
    x = rng.normal(size=(N, D)).astype(np.float32)
    K = plan["K"]
    g0 = plan["group_tile0"]
    for p in range(2):
        agg = np.zeros((NP, D), np.float64)
        gs = gstreams[p].astype(np.int64)
        dc = dstreams[p]
        for q in range(NCORES):
            for c in range(NCHUNK):
                for k in range(int(K[q, c])):
                    t = int(g0[q, c]) + k
                    g = x[q * NPART:(q + 1) * NPART][gs[t * 128:(t + 1) * 128]]
                    dr = dc[:, t]
                    msk = dr >= 0
                    np.add.at(agg, (c * 128 + dr[msk].astype(np.int64)), g[msk])
        sel = (dst >= p * NPART) & (dst < (p + 1) * NPART)
        ref = np.zeros((NPART, D), np.float64)
        np.add.at(ref, dst[sel] - p * NPART, x[src[sel]])
        err = np.abs(agg[:NPART] - ref).max()
        print(f"core {p}: plan-emulated agg err {err:.3e}")


# revision 9
# speedup vs baseline: 1.3967x; 1.3967x over previous
"""Trainium2 Bass kernel for a 2-layer GraphConv block (PyG GraphConv, aggr=add):
    h1  = leaky_relu(segsum(x[src], dst) @ W1_rel.T + b1 + x @ W1_root.T)
    out = leaky_relu(segsum(h1[src], dst) @ W2_rel.T + b2 + h1 @ W2_root.T + x)

Self-contained: takes full inputs, shards nodes (dst) across 8 NeuronCores,
runs one SPMD Bass program, returns the full output.

Aggregation strategy (v4): no DMA scatter-add. Edges are grouped by
(src part q, dst chunk c of 128 rows), gathered with swdge dma_gather into
SBUF tiles of 128 tokens, and summed into per-chunk PSUM accumulators with
PE matmuls against one-hot matrices S[j, m] = (dst_rel[j] == m) generated
on-chip (DVE is_equal / Act |x|+relu against iota constants). Group sizes
are padded to the max across cores (not 128-aligned) so the instruction
stream is uniform (SPMD); a 128-token tile may straddle two adjacent chunks
and then contributes one matmul to each (iota windows 0..127 / 128..255).
Each chunk's accumulation matmuls run consecutively into a bank-sized PSUM
accumulator (PSUM start=True zeroes a whole 2KB region, so groups cannot
interleave within a bank).
"""
import sys

sys.path.insert(0, '/opt/trn_rl_repo')

import numpy as np

N = 100000
D = 64
NCORES = 8
NPART = N // NCORES            # 12500
NP = 12544                     # 98 * 128, padded part size
NCHUNK = NP // 128             # 98
SBC = 8                        # chunks per superblock
NSB = (NCHUNK + SBC - 1) // SBC
NEG_SLOPE = 0.01


def _round128(n):
    return ((n + 127) // 128) * 128


def _make_plan(src, dst):
    """Uniform SPMD plan + per-core index/dst streams.

    Gather-stream order: superblock sb -> src part q -> chunk c; group (q, c)
    holds P[q, c] = max(128, max_core_count) tokens (exact, not 128-aligned);
    each (q, sb) range is padded to a 128 multiple.
    Returns (plan, gstreams, dstreams):
      plan: dict(L, T, calls, mms, group_pos)
        calls[sb][q] = (off_tokens, ntok, tile_lo) or None
        mms[sb] = [(c, tile, win, start, stop)] in issue order; win 0/1 selects
                  the iota window (c == c_lo(tile) + win)
      gstreams: int16 [NCORES, L] gather idx into part-q tensor (pad 0)
      dstreams: float32 [NCORES, 128, T] dst rel to 128*c_lo(tile) (pad -1)
    """
    per_core = []
    counts = np.zeros((NCORES, NCORES, NCHUNK), np.int64)
    for p in range(NCORES):
        sel = (dst >= p * NPART) & (dst < (p + 1) * NPART)
        s = src[sel]
        dloc = (dst[sel] - p * NPART).astype(np.int64)
        q = s // NPART
        sl = (s - q * NPART).astype(np.int64)
        c = dloc // 128
        dr = dloc - c * 128
        np.add.at(counts, (p, q, c), 1)
        per_core.append((q, c, sl, dr))

    P = np.maximum(counts.max(axis=0), 128)        # [q, c] tokens per group

    # stream layout: group start positions, call ranges, tile chunk windows
    pos = 0
    group_pos = np.zeros((NCORES, NCHUNK), np.int64)
    calls = []
    mms = []
    tile_clo = []                                  # c_lo per tile
    for sb in range(NSB):
        c0, c1 = sb * SBC, min((sb + 1) * SBC, NCHUNK)
        sb_calls = []
        chunk_mms = {c: [] for c in range(c0, c1)}
        for q in range(NCORES):
            off = pos
            bounds = []                            # (c, lo, hi) token ranges
            for c in range(c0, c1):
                group_pos[q, c] = pos
                bounds.append((c, pos, pos + int(P[q, c])))
                pos += int(P[q, c])
            pos = off + _round128(pos - off)
            ntok = pos - off
            tlo = len(tile_clo)
            nt = ntok // 128
            # per-tile chunk windows
            bi = 0
            for t in range(nt):
                a = off + t * 128
                while bi < len(bounds) - 1 and bounds[bi][2] <= a:
                    bi += 1
                c_lo = bounds[bi][0]
                tile_clo.append(c_lo)
                gt = tlo + t
                chunk_mms[c_lo].append((gt, 0))
                if bi < len(bounds) - 1 and bounds[bi + 1][1] < a + 128:
                    chunk_mms[c_lo + 1].append((gt, 1))
            sb_calls.append((off, ntok, tlo))
        calls.append(sb_calls)
        sb_mms = []
        for c in range(c0, c1):
            lst = chunk_mms[c]
            assert lst, f"chunk {c} got no matmuls"
            for i, (gt, win) in enumerate(lst):
                sb_mms.append((c, gt, win, i == 0, i == len(lst) - 1))
        mms.append(sb_mms)
    L = pos
    T = len(tile_clo)
    tile_clo = np.array(tile_clo)

    gstreams = np.zeros((NCORES, L), np.int16)
    dstreams = np.full((NCORES, 128, T), -1.0, np.float32)
    for p, (q, c, sl, dr) in enumerate(per_core):
        o = np.lexsort((sl, c, q))   # src-sorted within groups for DRAM locality
        q2, c2, sl2, dr2 = q[o], c[o], sl[o], dr[o]
        key = q2 * NCHUNK + c2
        nb = np.r_[True, key[1:] != key[:-1]]
        gid = np.cumsum(nb) - 1
        st = np.flatnonzero(nb)
        off_in_grp = np.arange(len(key)) - st[gid]
        posn = group_pos[q2[st], c2[st]][gid] + off_in_grp
        gstreams[p][posn] = sl2.astype(np.int16)
        # dst value relative to the containing tile's c_lo window
        rel = dr2 + 128 * (c2 - tile_clo[posn // 128])
        assert rel.min() >= 0 and rel.max() < 256
        dstreams[p][posn % 128, posn // 128] = rel

    plan = dict(L=L, T=T, calls=calls, mms=mms)
    return plan, gstreams, dstreams


def _wrap_stream(a):
    """[L] int16 -> [128, L//16] wrapped (idx i at [i%16, i//16]) replicated 8x."""
    L = len(a)
    assert L % 16 == 0
    w = a.reshape(L // 16, 16).T  # [16, cols]
    return np.tile(w, (8, 1)).copy()


def _build_nc(plan):
    from concourse import tile, mybir, masks
    import concourse.bacc as bacc

    L, T = plan["L"], plan["T"]
    cols = L // 16
    f32 = mybir.dt.float32
    bf16 = mybir.dt.bfloat16
    i16 = mybir.dt.int16

    # max tiles per gather call (for SBUF gather-buffer sizing)
    max_call_tiles = max(c[1] // 128 for sbc in plan["calls"] for c in sbc if c)
    max_sb_cols = max(sum(c[1] for c in sbc if c) // 16 for sbc in plan["calls"])

    nc = bacc.Bacc(None, target_bir_lowering=False, num_devices=NCORES,
                   dynamic_dma_scratch_size=16384, num_swdge_queues=4)

    x_parts = [nc.declare_dram_parameter(f"x_part{q}", [NP, D], f32, isOutput=False)
               for q in range(NCORES)]
    xT_in = nc.declare_dram_parameter("xT", [D, NP], f32, isOutput=False)
    w_ins = {}
    for nm in ["W1relT", "W1rootT", "W2relT", "W2rootT"]:
        w_ins[nm] = nc.declare_dram_parameter(nm, [D, D], f32, isOutput=False)
    b_ins = {nm: nc.declare_dram_parameter(nm, [1, D], f32, isOutput=False)
             for nm in ["b1", "b2"]}
    gidx_in = nc.declare_dram_parameter("gidx", [128, cols], i16, isOutput=False)
    dcol_in = nc.declare_dram_parameter("dcol", [128, T], f32, isOutput=False)
    ndcol_in = nc.declare_dram_parameter("ndcol", [128, T], f32, isOutput=False)
    iota_in = nc.declare_dram_parameter("iota", [128, 256], f32, isOutput=False)
    y_out = nc.declare_dram_parameter("y", [NP, D], f32, isOutput=True)

    h1_bounce = nc.dram_tensor("h1_bounce", [NP, D], f32)
    h_full = nc.dram_tensor("h_full", [NCORES * NP, D], f32, addr_space="Shared")

    with tile.TileContext(nc) as tc:
        with (
            tc.tile_pool(name="const", bufs=1) as cpool,
            tc.tile_pool(name="gi", bufs=2) as ipool,
            tc.tile_pool(name="gbuf", bufs=4) as gpool,
            tc.tile_pool(name="g16", bufs=2) as g16pool,
            tc.tile_pool(name="sgen", bufs=6) as spool,
            tc.tile_pool(name="mm", bufs=3) as mpool,
            tc.tile_pool(name="acc", bufs=3, space="PSUM") as apool,
            tc.tile_pool(name="psmall", bufs=2, space="PSUM") as ppool,
        ):
            # ---- constants ----
            ident = cpool.tile([128, 128], f32)
            masks.make_identity(nc, ident[:])
            ones1 = cpool.tile([1, 128], f32)
            nc.gpsimd.memset(ones1[:], 1.0)
            iota_t = cpool.tile([128, 256], f32)
            nc.sync.dma_start(iota_t[:], iota_in[:])
            wt = {}
            for nm, t_in in w_ins.items():
                t = cpool.tile([D, D], f32, tag=nm)
                nc.sync.dma_start(t[:], t_in[:])
                wt[nm] = t
            bt = {}
            for nm, t_in in b_ins.items():
                t = cpool.tile([1, D], f32, tag=nm)
                nc.sync.dma_start(t[:], t_in[:])
                bt[nm] = t

            # ---- resident streams / features ----
            dcol = cpool.tile([128, T], f32)
            nc.sync.dma_start(dcol[:], dcol_in[:])
            ndcol = cpool.tile([128, T], f32)
            nc.sync.dma_start(ndcol[:], ndcol_in[:])
            xT_sb = cpool.tile([D, NP], f32)
            for a in range(0, NP, 4096):
                n = min(4096, NP - a)
                nc.sync.dma_start(xT_sb[:, a:a + n], xT_in[:, a:a + n])
            h1T_sb = cpool.tile([D, NP], f32)

            qn = [0]
            sgen_n = [0]

            def make_S(t, win):
                S = spool.tile([128, 128], bf16, tag="S")
                sgen_n[0] += 1
                if sgen_n[0] % 2 == 0:
                    nc.vector.tensor_scalar(
                        S[:], iota_t[:, 128 * win:128 * (win + 1)],
                        dcol[:, t:t + 1], None,
                        op0=mybir.AluOpType.is_equal)
                else:
                    sa = spool.tile([128, 128], f32, tag="sabs")
                    nc.scalar.activation(
                        sa[:], iota_t[:, 128 * win:128 * (win + 1)],
                        mybir.ActivationFunctionType.Abs,
                        bias=ndcol[:, t:t + 1])
                    nc.scalar.activation(
                        S[:], sa[:],
                        mybir.ActivationFunctionType.Relu,
                        bias=1.0, scale=-1.0)
                return S

            def layer(src_aps, w_rel, w_root, bias, root_sb, residual,
                      out_rows, save_h1T):
                for sb in range(NSB):
                    c0, c1 = sb * SBC, min((sb + 1) * SBC, NCHUNK)
                    metas = [m for m in plan["calls"][sb] if m is not None]
                    sb_off = min(m[0] for m in metas)
                    sb_end = max(m[0] + m[1] for m in metas)
                    gi = ipool.tile([128, max_sb_cols], i16, tag="gi")
                    nc.sync.dma_start(gi[:, :(sb_end - sb_off) // 16],
                                      gidx_in[:, sb_off // 16:sb_end // 16])
                    bufs = {}
                    for q in range(NCORES):
                        meta = plan["calls"][sb][q]
                        if meta is None:
                            continue
                        off, ntok, tlo = meta
                        rows = ntok // 128
                        gb = gpool.tile([128, max_call_tiles, D], f32, tag="gb")
                        # swdge gather calls are capped at 1024 idxs (64
                        # descs/engine single-packet limit) -- split.
                        for a in range(0, rows, 8):
                            r = min(8, rows - a)
                            o16 = (off - sb_off) // 16
                            nc.gpsimd.dma_gather(
                                gb[:, a:a + r, :], src_aps[q],
                                gi[:, o16 + a * 8:o16 + (a + r) * 8],
                                r * 128, r * 128, D, queue_num=qn[0] % 4)
                            qn[0] += 1
                        gb16 = g16pool.tile([128, max_call_tiles, D], bf16,
                                            tag=f"g16_{q}")
                        nc.vector.tensor_copy(gb16[:, :rows, :], gb[:, :rows, :])
                        bufs[q] = (gb16, tlo)
                    # chunk-major accumulation
                    tile_of = {}
                    for q in range(NCORES):
                        meta = plan["calls"][sb][q]
                        if meta is None:
                            continue
                        _, ntok, tlo = meta
                        for t in range(tlo, tlo + ntok // 128):
                            tile_of[t] = (bufs[q][0], t - tlo)
                    cur_c = None
                    acc = None
                    accs = {}
                    for (c, gt, win, st, sp) in plan["mms"][sb]:
                        if st:
                            accs[c] = apool.tile([128, D], f32, tag="acc",
                                                 name=f"acc{c}")
                        acc = accs[c]
                        gb16, t_loc = tile_of[gt]
                        S = make_S(gt, win)
                        nc.tensor.matmul(
                            acc[:], S[:], gb16[:, t_loc, :],
                            start=st, stop=sp)
                        if not sp:
                            continue
                        # dense tail for chunk c
                        agg_sb = mpool.tile([128, D], f32, tag="agg")
                        nc.vector.tensor_copy(agg_sb[:], acc[:])
                        ps_t = ppool.tile([D, 128], f32, tag="ps_t")
                        nc.tensor.matmul(ps_t[:], agg_sb[:], ident[:, :128],
                                         is_transpose=True)
                        aT = mpool.tile([D, 128], f32, tag="aT")
                        nc.vector.tensor_copy(aT[:], ps_t[:])
                        po = ppool.tile([128, D], f32, tag="po")
                        nc.tensor.matmul(po[:], aT[:], w_rel[:],
                                         start=True, stop=False)
                        nc.tensor.matmul(po[:], root_sb[:, 128 * c:128 * (c + 1)],
                                         w_root[:], start=False, stop=False)
                        if residual:
                            nc.tensor.matmul(po[:], xT_sb[:, 128 * c:128 * (c + 1)],
                                             ident[:D, :D], start=False, stop=False)
                        nc.tensor.matmul(po[:], ones1[:], bias[:],
                                         start=False, stop=True)
                        tmp = mpool.tile([128, D], f32, tag="tmp")
                        nc.vector.tensor_scalar_mul(tmp[:], po[:], NEG_SLOPE)
                        hrow = mpool.tile([128, D], f32, tag="hrow")
                        nc.vector.tensor_max(hrow[:], po[:], tmp[:])
                        nc.sync.dma_start(out_rows[128 * c:128 * (c + 1), :],
                                          hrow[:])
                        if save_h1T:
                            ps_h = ppool.tile([D, 128], f32, tag="ps_t")
                            nc.tensor.matmul(ps_h[:], hrow[:], ident[:, :128],
                                             is_transpose=True)
                            nc.vector.tensor_copy(h1T_sb[:, 128 * c:128 * (c + 1)],
                                                  ps_h[:])

            # ================= layer 1 =================
            layer([xp[:] for xp in x_parts], wt["W1relT"], wt["W1rootT"],
                  bt["b1"], xT_sb, False, h1_bounce, True)

            # ================= halo exchange =================
            nc.gpsimd.collective_compute(
                "AllGather", mybir.AluOpType.bypass,
                replica_groups=[list(range(NCORES))],
                ins=[h1_bounce[:].opt()], outs=[h_full[:].opt()])

            # ================= layer 2 =================
            layer([h_full[q * NP:(q + 1) * NP, :] for q in range(NCORES)],
                  wt["W2relT"], wt["W2rootT"], bt["b2"], h1T_sb, True,
                  y_out, False)

    nc.compile()
    return nc


def _prep_inputs(x, edge_index, W1_rel, b1, W1_root, W2_rel, b2, W2_root):
    src = np.asarray(edge_index[0]).astype(np.int64)
    dst = np.asarray(edge_index[1]).astype(np.int64)
    plan, gstreams, dstreams = _make_plan(src, dst)

    x = np.asarray(x, np.float32)
    xp_all = []
    for q in range(NCORES):
        xp = np.zeros((NP, D), np.float32)
        xp[:NPART] = x[q * NPART:(q + 1) * NPART]
        xp_all.append(xp)

    common = {f"x_part{q}": xp_all[q] for q in range(NCORES)}
    common["W1relT"] = np.ascontiguousarray(np.asarray(W1_rel, np.float32).T)
    common["W1rootT"] = np.ascontiguousarray(np.asarray(W1_root, np.float32).T)
    common["W2relT"] = np.ascontiguousarray(np.asarray(W2_rel, np.float32).T)
    common["W2rootT"] = np.ascontiguousarray(np.asarray(W2_root, np.float32).T)
    common["b1"] = np.asarray(b1, np.float32).reshape(1, D)
    common["b2"] = np.asarray(b2, np.float32).reshape(1, D)
    common["iota"] = np.tile(np.arange(256, dtype=np.float32)[None, :],
                             (128, 1))

    in_maps = []
    for p in range(NCORES):
        m = dict(common)
        m["xT"] = np.ascontiguousarray(xp_all[p].T)
        m["gidx"] = _wrap_stream(gstreams[p])
        m["dcol"] = dstreams[p]
        m["ndcol"] = -dstreams[p]
        in_maps.append(m)
    return plan, in_maps


def kernel(x, edge_index, W1_rel, b1, W1_root, W2_rel, b2, W2_root):
    from concourse import bass_utils

    plan, in_maps = _prep_inputs(x, edge_index, W1_rel, b1, W1_root,
                                 W2_rel, b2, W2_root)
    nc = _build_nc(plan)
    res = bass_utils.run_bass_kernel_spmd(nc, in_maps, core_ids=list(range(NCORES)))
    out = np.concatenate([res.results[p]["y"][:NPART] for p in range(NCORES)], 0)
    return out.astype(np.float32)


if __name__ == "__main__":
    # host-side plan self-check in numpy (no device)
    rng = np.random.default_rng(0)
    E = 1600000
    src = rng.integers(0, N, E)
    dst = rng.integers(0, N, E)
    plan, gstreams, dstreams = _make_plan(src, dst)
    nmm = sum(len(m) for m in plan["mms"])
    print(f"L={plan['L']} T={plan['T']} mms={nmm} "
          f"maxcall={max(c[1]//128 for sbc in plan['calls'] for c in sbc if c)}")
    x = rng.normal(size=(N, D)).astype(np.float32)
    # reconstruct tile c_lo from mms (win=0 entries)
    clo = {}
    for sbm in plan["mms"]:
        for (c, gt, win, st, sp) in sbm:
            if win == 0:
                clo[gt] = c
    for p in range(2):
        agg = np.zeros((NP + 256, D), np.float64)
        gs = gstreams[p].astype(np.int64)
        dc = dstreams[p]
        for t in range(plan["T"]):
            g = None
            # find source part q for tile t
            for sb in range(NSB):
                for q in range(NCORES):
                    meta = plan["calls"][sb][q]
                    if meta and meta[2] <= t < meta[2] + meta[1] // 128:
                        off = meta[0] + (t - meta[2]) * 128
                        g = x[q * NPART:(q + 1) * NPART][gs[off:off + 128]]
            dr = dc[:, t]
            msk = dr >= 0
            np.add.at(agg, (clo[t] * 128 + dr[msk].astype(np.int64)), g[msk])
        sel = (dst >= p * NPART) & (dst < (p + 1) * NPART)
        ref = np.zeros((NPART, D), np.float64)
        np.add.at(ref, dst[sel] - p * NPART, x[src[sel]])
        err = np.abs(agg[:NPART] - ref).max()
        print(f"core {p}: plan-emulated agg err {err:.3e}")


# revision 11
# speedup vs baseline: 1.6236x; 1.1625x over previous
"""Trainium2 Bass kernel for a 2-layer GraphConv block (PyG GraphConv, aggr=add):
    h1  = leaky_relu(segsum(x[src], dst) @ W1_rel.T + b1 + x @ W1_root.T)
    out = leaky_relu(segsum(h1[src], dst) @ W2_rel.T + b2 + h1 @ W2_root.T + x)

Self-contained: takes full inputs, shards nodes (dst) across 8 NeuronCores,
runs one SPMD Bass program, returns the full output.

Aggregation strategy (v4): no DMA scatter-add. Edges are grouped by
(src part q, dst chunk c of 128 rows), gathered with swdge dma_gather into
SBUF tiles of 128 tokens, and summed into per-chunk PSUM accumulators with
PE matmuls against one-hot matrices S[j, m] = (dst_rel[j] == m) generated
on-chip (DVE is_equal / Act |x|+relu against iota constants). Group sizes
are padded to the max across cores (not 128-aligned) so the instruction
stream is uniform (SPMD); a 128-token tile may straddle two adjacent chunks
and then contributes one matmul to each (iota windows 0..127 / 128..255).
Each chunk's accumulation matmuls run consecutively into a bank-sized PSUM
accumulator (PSUM start=True zeroes a whole 2KB region, so groups cannot
interleave within a bank).
"""
import sys

sys.path.insert(0, '/opt/trn_rl_repo')

import numpy as np

N = 100000
D = 64
NCORES = 8
NPART = N // NCORES            # 12500
NP = 12544                     # 98 * 128, padded part size
NCHUNK = NP // 128             # 98
SBC = 8                        # chunks per superblock
NSB = (NCHUNK + SBC - 1) // SBC
NEG_SLOPE = 0.01


def _round128(n):
    return ((n + 127) // 128) * 128


def _make_plan(src, dst):
    """Uniform SPMD plan + per-core index/dst streams.

    Gather-stream order: superblock sb -> src part q -> chunk c; group (q, c)
    holds P[q, c] = max(128, max_core_count) tokens (exact, not 128-aligned);
    each (q, sb) range is padded to a 128 multiple.
    Returns (plan, gstreams, dstreams):
      plan: dict(L, T, calls, mms, group_pos)
        calls[sb][q] = (off_tokens, ntok, tile_lo) or None
        mms[sb] = [(c, tile, win, start, stop)] in issue order; win 0/1 selects
                  the iota window (c == c_lo(tile) + win)
      gstreams: int16 [NCORES, L] gather idx into part-q tensor (pad 0)
      dstreams: float32 [NCORES, 128, T] dst rel to 128*c_lo(tile) (pad -1)
    """
    per_core = []
    counts = np.zeros((NCORES, NCORES, NCHUNK), np.int64)
    for p in range(NCORES):
        sel = (dst >= p * NPART) & (dst < (p + 1) * NPART)
        s = src[sel]
        dloc = (dst[sel] - p * NPART).astype(np.int64)
        q = s // NPART
        sl = (s - q * NPART).astype(np.int64)
        c = dloc // 128
        dr = dloc - c * 128
        np.add.at(counts, (p, q, c), 1)
        per_core.append((q, c, sl, dr))

    P = np.maximum(counts.max(axis=0), 128)        # [q, c] tokens per group

    # stream layout: group start positions, call ranges, tile chunk windows
    pos = 0
    group_pos = np.zeros((NCORES, NCHUNK), np.int64)
    calls = []
    mms = []
    tile_clo = []                                  # c_lo per tile
    for sb in range(NSB):
        c0, c1 = sb * SBC, min((sb + 1) * SBC, NCHUNK)
        sb_calls = []
        chunk_mms = {c: [] for c in range(c0, c1)}
        for q in range(NCORES):
            off = pos
            bounds = []                            # (c, lo, hi) token ranges
            for c in range(c0, c1):
                group_pos[q, c] = pos
                bounds.append((c, pos, pos + int(P[q, c])))
                pos += int(P[q, c])
            pos = off + _round128(pos - off)
            ntok = pos - off
            tlo = len(tile_clo)
            nt = ntok // 128
            # per-tile chunk windows
            bi = 0
            for t in range(nt):
                a = off + t * 128
                while bi < len(bounds) - 1 and bounds[bi][2] <= a:
                    bi += 1
                c_lo = bounds[bi][0]
                tile_clo.append(c_lo)
                gt = tlo + t
                chunk_mms[c_lo].append((gt, 0))
                if bi < len(bounds) - 1 and bounds[bi + 1][1] < a + 128:
                    chunk_mms[c_lo + 1].append((gt, 1))
            sb_calls.append((off, ntok, tlo))
        calls.append(sb_calls)
        sb_mms = []
        for c in range(c0, c1):
            lst = chunk_mms[c]
            assert lst, f"chunk {c} got no matmuls"
            for i, (gt, win) in enumerate(lst):
                sb_mms.append((c, gt, win, i == 0, i == len(lst) - 1))
        mms.append(sb_mms)
    L = pos
    T = len(tile_clo)
    tile_clo = np.array(tile_clo)

    gstreams = np.zeros((NCORES, L), np.int16)
    dstreams = np.full((NCORES, 128, T), -1.0, np.float32)
    for p, (q, c, sl, dr) in enumerate(per_core):
        o = np.lexsort((sl, c, q))   # src-sorted within groups for DRAM locality
        q2, c2, sl2, dr2 = q[o], c[o], sl[o], dr[o]
        key = q2 * NCHUNK + c2
        nb = np.r_[True, key[1:] != key[:-1]]
        gid = np.cumsum(nb) - 1
        st = np.flatnonzero(nb)
        off_in_grp = np.arange(len(key)) - st[gid]
        posn = group_pos[q2[st], c2[st]][gid] + off_in_grp
        gstreams[p][posn] = sl2.astype(np.int16)
        # dst value relative to the containing tile's c_lo window
        rel = dr2 + 128 * (c2 - tile_clo[posn // 128])
        assert rel.min() >= 0 and rel.max() < 256
        dstreams[p][posn % 128, posn // 128] = rel

    plan = dict(L=L, T=T, calls=calls, mms=mms)
    return plan, gstreams, dstreams


def _wrap_stream(a):
    """[L] int16 -> [128, L//16] wrapped (idx i at [i%16, i//16]) replicated 8x."""
    L = len(a)
    assert L % 16 == 0
    w = a.reshape(L // 16, 16).T  # [16, cols]
    return np.tile(w, (8, 1)).copy()


def _build_nc(plan):
    from concourse import tile, mybir, masks
    import concourse.bacc as bacc

    L, T = plan["L"], plan["T"]
    cols = L // 16
    f32 = mybir.dt.float32
    bf16 = mybir.dt.bfloat16
    i16 = mybir.dt.int16

    # max tiles per gather call (for SBUF gather-buffer sizing)
    max_call_tiles = max(c[1] // 128 for sbc in plan["calls"] for c in sbc if c)
    max_sb_cols = max(sum(c[1] for c in sbc if c) // 16 for sbc in plan["calls"])

    nc = bacc.Bacc(None, target_bir_lowering=False, num_devices=NCORES,
                   dynamic_dma_scratch_size=16384, num_swdge_queues=4)

    x_parts = [nc.declare_dram_parameter(f"x_part{q}", [NP, 128], bf16,
                                         isOutput=False)
               for q in range(NCORES)]
    xT_in = nc.declare_dram_parameter("xT", [D, NP], f32, isOutput=False)
    w_ins = {}
    for nm in ["W1relT", "W1rootT", "W2relT", "W2rootT"]:
        w_ins[nm] = nc.declare_dram_parameter(nm, [D, D], f32, isOutput=False)
    b_ins = {nm: nc.declare_dram_parameter(nm, [1, D], f32, isOutput=False)
             for nm in ["b1", "b2"]}
    gidx_in = nc.declare_dram_parameter("gidx", [128, cols], i16, isOutput=False)
    dcol_in = nc.declare_dram_parameter("dcol", [128, T], f32, isOutput=False)
    ndcol_in = nc.declare_dram_parameter("ndcol", [128, T], f32, isOutput=False)
    iota_in = nc.declare_dram_parameter("iota", [128, 256], f32, isOutput=False)
    y_out = nc.declare_dram_parameter("y", [NP, D], f32, isOutput=True)

    h1_bounce = nc.dram_tensor("h1_bounce", [NP, 128], bf16)
    h_full = nc.dram_tensor("h_full", [NCORES * NP, 128], bf16,
                            addr_space="Shared")

    with tile.TileContext(nc) as tc:
        with (
            tc.tile_pool(name="const", bufs=1) as cpool,
            tc.tile_pool(name="gi", bufs=2) as ipool,
            tc.tile_pool(name="g16", bufs=2) as g16pool,
            tc.tile_pool(name="sgen", bufs=6) as spool,
            tc.tile_pool(name="mm", bufs=3) as mpool,
            tc.tile_pool(name="acc", bufs=3, space="PSUM") as apool,
            tc.tile_pool(name="psmall", bufs=2, space="PSUM") as ppool,
        ):
            # ---- constants ----
            ident = cpool.tile([128, 128], f32)
            masks.make_identity(nc, ident[:])
            ones1 = cpool.tile([1, 128], f32)
            nc.gpsimd.memset(ones1[:], 1.0)
            iota_t0 = cpool.tile([128, 128], f32)
            nc.sync.dma_start(iota_t0[:], iota_in[:, 0:128])
            iota_t1 = cpool.tile([128, 128], f32)
            nc.sync.dma_start(iota_t1[:], iota_in[:, 128:256])
            iotas = [iota_t0, iota_t1]
            wt = {}
            for nm, t_in in w_ins.items():
                t = cpool.tile([D, D], f32, tag=nm)
                nc.sync.dma_start(t[:], t_in[:])
                wt[nm] = t
            bt = {}
            for nm, t_in in b_ins.items():
                t = cpool.tile([1, D], f32, tag=nm)
                nc.sync.dma_start(t[:], t_in[:])
                bt[nm] = t

            # ---- resident streams / features ----
            dcol = cpool.tile([128, T], f32)
            nc.sync.dma_start(dcol[:], dcol_in[:])
            ndcol = cpool.tile([128, T], f32)
            nc.sync.dma_start(ndcol[:], ndcol_in[:])
            xT_sb = cpool.tile([D, NP], f32)
            for a in range(0, NP, 4096):
                n = min(4096, NP - a)
                nc.sync.dma_start(xT_sb[:, a:a + n], xT_in[:, a:a + n])
            h1T_sb = cpool.tile([D, NP], f32)

            qn = [0]
            sgen_n = [0]

            def make_S(t, win):
                S = spool.tile([128, 128], bf16, tag="S")
                sgen_n[0] += 1
                if sgen_n[0] % 3 != 0:
                    nc.vector.tensor_scalar(
                        S[:], iotas[win][:], dcol[:, t:t + 1], None,
                        op0=mybir.AluOpType.is_equal)
                else:
                    sa = spool.tile([128, 128], f32, tag="sabs")
                    nc.scalar.activation(
                        sa[:], iotas[win][:],
                        mybir.ActivationFunctionType.Abs,
                        bias=ndcol[:, t:t + 1])
                    nc.scalar.activation(
                        S[:], sa[:],
                        mybir.ActivationFunctionType.Relu,
                        bias=1.0, scale=-1.0)
                return S

            def layer(src_aps, w_rel, w_root, bias, root_sb, residual,
                      out_rows, save_h1T):
                for sb in range(NSB):
                    c0, c1 = sb * SBC, min((sb + 1) * SBC, NCHUNK)
                    metas = [m for m in plan["calls"][sb] if m is not None]
                    sb_off = min(m[0] for m in metas)
                    sb_end = max(m[0] + m[1] for m in metas)
                    gi = ipool.tile([128, max_sb_cols], i16, tag="gi")
                    nc.sync.dma_start(gi[:, :(sb_end - sb_off) // 16],
                                      gidx_in[:, sb_off // 16:sb_end // 16])
                    bufs = {}
                    for q in range(NCORES):
                        meta = plan["calls"][sb][q]
                        if meta is None:
                            continue
                        off, ntok, tlo = meta
                        rows = ntok // 128
                        gb16 = g16pool.tile([128, max_call_tiles, 128], bf16,
                                            tag=f"g16_{q}")
                        # swdge gather calls are capped at 1024 idxs (64
                        # descs/engine single-packet limit) -- split.
                        for a in range(0, rows, 8):
                            r = min(8, rows - a)
                            o16 = (off - sb_off) // 16
                            nc.gpsimd.dma_gather(
                                gb16[:, a:a + r, :], src_aps[q],
                                gi[:, o16 + a * 8:o16 + (a + r) * 8],
                                r * 128, r * 128, 128, queue_num=qn[0] % 4)
                            qn[0] += 1
                        bufs[q] = (gb16, tlo)
                    # chunk-major accumulation
                    tile_of = {}
                    for q in range(NCORES):
                        meta = plan["calls"][sb][q]
                        if meta is None:
                            continue
                        _, ntok, tlo = meta
                        for t in range(tlo, tlo + ntok // 128):
                            tile_of[t] = (bufs[q][0], t - tlo)
                    cur_c = None
                    acc = None
                    accs = {}
                    for (c, gt, win, st, sp) in plan["mms"][sb]:
                        if st:
                            accs[c] = apool.tile([128, D], f32, tag="acc",
                                                 name=f"acc{c}")
                        acc = accs[c]
                        gb16, t_loc = tile_of[gt]
                        S = make_S(gt, win)
                        nc.tensor.matmul(
                            acc[:], S[:], gb16[:, t_loc, 0:D],
                            start=st, stop=sp)
                        if not sp:
                            continue
                        # dense tail for chunk c
                        agg_sb = mpool.tile([128, D], f32, tag="agg")
                        nc.vector.tensor_copy(agg_sb[:], acc[:])
                        ps_t = ppool.tile([D, 128], f32, tag="ps_t")
                        nc.tensor.matmul(ps_t[:], agg_sb[:], ident[:, :128],
                                         is_transpose=True)
                        aT = mpool.tile([D, 128], f32, tag="aT")
                        nc.vector.tensor_copy(aT[:], ps_t[:])
                        po = ppool.tile([128, D], f32, tag="po")
                        nc.tensor.matmul(po[:], aT[:], w_rel[:],
                                         start=True, stop=False)
                        nc.tensor.matmul(po[:], root_sb[:, 128 * c:128 * (c + 1)],
                                         w_root[:], start=False, stop=False)
                        if residual:
                            nc.tensor.matmul(po[:], xT_sb[:, 128 * c:128 * (c + 1)],
                                             ident[:D, :D], start=False, stop=False)
                        nc.tensor.matmul(po[:], ones1[:], bias[:],
                                         start=False, stop=True)
                        tmp = mpool.tile([128, D], f32, tag="tmp")
                        nc.vector.tensor_scalar_mul(tmp[:], po[:], NEG_SLOPE)
                        hrow = mpool.tile([128, D], f32, tag="hrow")
                        nc.vector.tensor_max(hrow[:], po[:], tmp[:])
                        if save_h1T:
                            # layer 1: store bf16 padded rows (pad cols are
                            # garbage; layer-2 matmuls only read cols 0:64)
                            hrow16 = mpool.tile([128, 128], bf16, tag="hrow16")
                            nc.gpsimd.memset(hrow16[:, D:], 0)
                            nc.vector.tensor_copy(hrow16[:, :D], hrow[:])
                            nc.sync.dma_start(
                                out_rows[128 * c:128 * (c + 1), :], hrow16[:])
                            ps_h = ppool.tile([D, 128], f32, tag="ps_t")
                            nc.tensor.matmul(ps_h[:], hrow[:], ident[:, :128],
                                             is_transpose=True)
                            nc.vector.tensor_copy(h1T_sb[:, 128 * c:128 * (c + 1)],
                                                  ps_h[:])
                        else:
                            nc.sync.dma_start(
                                out_rows[128 * c:128 * (c + 1), :], hrow[:])

            # ================= layer 1 =================
            layer([xp[:] for xp in x_parts], wt["W1relT"], wt["W1rootT"],
                  bt["b1"], xT_sb, False, h1_bounce, True)

            # ================= halo exchange =================
            nc.gpsimd.collective_compute(
                "AllGather", mybir.AluOpType.bypass,
                replica_groups=[list(range(NCORES))],
                ins=[h1_bounce[:].opt()], outs=[h_full[:].opt()])

            # ================= layer 2 =================
            layer([h_full[q * NP:(q + 1) * NP, :] for q in range(NCORES)],
                  wt["W2relT"], wt["W2rootT"], bt["b2"], h1T_sb, True,
                  y_out, False)

    nc.compile()
    return nc


def _prep_inputs(x, edge_index, W1_rel, b1, W1_root, W2_rel, b2, W2_root):
    src = np.asarray(edge_index[0]).astype(np.int64)
    dst = np.asarray(edge_index[1]).astype(np.int64)
    plan, gstreams, dstreams = _make_plan(src, dst)

    import ml_dtypes
    x = np.asarray(x, np.float32)
    xp_all = []
    xp16_all = []
    for q in range(NCORES):
        xp = np.zeros((NP, D), np.float32)
        xp[:NPART] = x[q * NPART:(q + 1) * NPART]
        xp_all.append(xp)
        xp16 = np.zeros((NP, 128), ml_dtypes.bfloat16)
        xp16[:, :D] = xp.astype(ml_dtypes.bfloat16)
        xp16_all.append(xp16)

    common = {f"x_part{q}": xp16_all[q] for q in range(NCORES)}
    common["W1relT"] = np.ascontiguousarray(np.asarray(W1_rel, np.float32).T)
    common["W1rootT"] = np.ascontiguousarray(np.asarray(W1_root, np.float32).T)
    common["W2relT"] = np.ascontiguousarray(np.asarray(W2_rel, np.float32).T)
    common["W2rootT"] = np.ascontiguousarray(np.asarray(W2_root, np.float32).T)
    common["b1"] = np.asarray(b1, np.float32).reshape(1, D)
    common["b2"] = np.asarray(b2, np.float32).reshape(1, D)
    common["iota"] = np.tile(np.arange(256, dtype=np.float32)[None, :],
                             (128, 1))

    in_maps = []
    for p in range(NCORES):
        m = dict(common)
        m["xT"] = np.ascontiguousarray(xp_all[p].T)
        m["gidx"] = _wrap_stream(gstreams[p])
        m["dcol"] = dstreams[p]
        m["ndcol"] = -dstreams[p]
        in_maps.append(m)
    return plan, in_maps


def kernel(x, edge_index, W1_rel, b1, W1_root, W2_rel, b2, W2_root):
    from concourse import bass_utils

    plan, in_maps = _prep_inputs(x, edge_index, W1_rel, b1, W1_root,
                                 W2_rel, b2, W2_root)
    nc = _build_nc(plan)
    res = bass_utils.run_bass_kernel_spmd(nc, in_maps, core_ids=list(range(NCORES)))
    out = np.concatenate([res.results[p]["y"][:NPART] for p in range(NCORES)], 0)
    return out.astype(np.float32)


if __name__ == "__main__":
    # host-side plan self-check in numpy (no device)
    rng = np.random.default_rng(0)
    E = 1600000
    src = rng.integers(0, N, E)
    dst = rng.integers(0, N, E)
    plan, gstreams, dstreams = _make_plan(src, dst)
    nmm = sum(len(m) for m in plan["mms"])
    print(f"L={plan['L']} T={plan['T']} mms={nmm} "
          f"maxcall={max(c[1]//128 for sbc in plan['calls'] for c in sbc if c)}")
    x = rng.normal(size=(N, D)).astype(np.float32)
    # reconstruct tile c_lo from mms (win=0 entries)
    clo = {}
    for sbm in plan["mms"]:
        for (c, gt, win, st, sp) in sbm:
            if win == 0:
                clo[gt] = c
    for p in range(2):
        agg = np.zeros((NP + 256, D), np.float64)
        gs = gstreams[p].astype(np.int64)
        dc = dstreams[p]
        for t in range(plan["T"]):
            g = None
            # find source part q for tile t
            for sb in range(NSB):
                for q in range(NCORES):
                    meta = plan["calls"][sb][q]
                    if meta and meta[2] <= t < meta[2] + meta[1] // 128:
                        off = meta[0] + (t - meta[2]) * 128
                        g = x[q * NPART:(q + 1) * NPART][gs[off:off + 128]]
            dr = dc[:, t]
            msk = dr >= 0
            np.add.at(agg, (clo[t] * 128 + dr[msk].astype(np.int64)), g[msk])
        sel = (dst >= p * NPART) & (dst < (p + 1) * NPART)
        ref = np.zeros((NPART, D), np.float64)
        np.add.at(ref, dst[sel] - p * NPART, x[src[sel]])
        err = np.abs(agg[:NPART] - ref).max()
        print(f"core {p}: plan-emulated agg err {err:.3e}")
